# revision 23
# baseline (speedup 1.0000x reference)
"""Trainium2 Bass kernel for an AttentionBlock (GroupNorm + single-head-dim
self-attention + proj + residual), data-parallel over batch on 8 NeuronCores.

Reference semantics (per batch element, x: [C=512, H=32, W=32], n = H*W = 1024):
  h   = GroupNorm32(x) * scale + bias
  q   = Wq h + bq ; k = Wk h + bk ; v = Wv h + bv     (1x1 convs, [C, n])
  S_h = q_h^T k_h / sqrt(64)   per head h (8 heads, d=64)
  A_h = softmax(S_h)           (over keys)
  o_h = v_h A_h^T
  y   = x + Wp o + bp

Sharding: batch 16 -> 2 per core, fully independent (no collectives).
"""

import numpy as np

import concourse.bacc as bacc
import concourse.bass as bass
import concourse.tile as tile
from concourse import mybir
from concourse.bass_utils import run_bass_kernel_spmd

F32 = mybir.dt.float32
F32R = mybir.dt.float32r
BF16 = mybir.dt.bfloat16
AF = mybir.ActivationFunctionType
OP = mybir.AluOpType

C = 512
NH = 8
D = 64
N = 1024
GROUPS = 32
GS = C // GROUPS  # 16 channels per group
EPS = 1e-5
B_PER_CORE = 2
N_CORES = 8

CT = 4   # c tiles of 128
NT = 8   # n tiles of 128
NCH = 2  # n chunks of 512
VG = 66  # vT per-head group stride (64 data + 1 ones + 1 pad)

E_BUFS = 10


def _bcast_rows(row_ap, parts):
    """Broadcast a single-row (DRAM) AP across `parts` partitions."""
    ap = [[0, parts]] + [list(d) for d in row_ap.ap]
    return bass.AP(tensor=row_ap.tensor, offset=row_ap.offset, ap=ap)


def build_nc(apply_vb, dump=False):
    nc = bacc.Bacc()

    x_ext = nc.declare_dram_parameter("x", [B_PER_CORE, 128, CT, N], F32, isOutput=False)
    w_ext = {}
    b_ext = {}
    for nm in ("q", "k", "v", "p"):
        w_ext[nm] = nc.declare_dram_parameter(f"{nm}wT", [128, CT, C], BF16, isOutput=False)
        b_ext[nm] = nc.declare_dram_parameter(f"{nm}b", [C], F32, isOutput=False)
    # packed per-channel vectors, pre-transposed host-side:
    # [128, 5, CT] = (nsc, nbi, qb, kb, pb) x c-tile
    vecs_ext = nc.declare_dram_parameter("vecs", [128, 5, CT], F32, isOutput=False)
    selr_ext = nc.declare_dram_parameter("selr", [128, CT, GROUPS], BF16, isOutput=False)
    sele_ext = nc.declare_dram_parameter("sele", [GROUPS, CT, 128], BF16, isOutput=False)
    out_ext = nc.declare_dram_parameter("out", [B_PER_CORE, 128, CT, N], F32, isOutput=True)

    zdram = nc.dram_tensor("zscratch", [B_PER_CORE, NH, N], BF16)
    dbg_ext = None
    if dump:
        dbg_ext = nc.declare_dram_parameter("dbg", [10, 128, 4352], F32, isOutput=True)

    with tile.TileContext(nc) as tc:
        with (
            tc.tile_pool(name="const", bufs=1) as const,
            tc.tile_pool(name="work", bufs=2) as work,
            tc.tile_pool(name="xpool", bufs=2) as xpool,
            tc.tile_pool(name="epool", bufs=E_BUFS) as epool,
            tc.tile_pool(name="small", bufs=2) as small,
            tc.tile_pool(name="ps1", bufs=1, space="PSUM") as ps1,
            tc.tile_pool(name="ps2", bufs=2, space="PSUM") as ps2,
        ):
            # ---- persistent weight / bias tiles -------------------------
            w_sb = {}
            for nm, eng in (("q", nc.sync), ("k", nc.sync), ("v", nc.sync), ("p", nc.sync)):
                w_sb[nm] = const.tile([128, CT, C], BF16, name=f"w_{nm}")
                eng.dma_start(out=w_sb[nm], in_=w_ext[nm].ap())
            vecs_sb = const.tile([128, 5, CT], F32)
            nc.sync.dma_start(out=vecs_sb, in_=vecs_ext.ap())
            nsc_sb = vecs_sb[:, 0, :]
            nbi_sb = vecs_sb[:, 1, :]
            bias_sb = {"q": vecs_sb[:, 2, :], "k": vecs_sb[:, 3, :], "p": vecs_sb[:, 4, :]}
            selr_sb = const.tile([128, CT, GROUPS], BF16)
            nc.sync.dma_start(out=selr_sb, in_=selr_ext.ap())
            sele_sb = const.tile([GROUPS, CT, 128], BF16)
            nc.sync.dma_start(out=sele_sb, in_=sele_ext.ap())
            vb_bc = None
            if apply_vb:
                vb_bc = const.tile([128, C], F32)
                nc.sync.dma_start(out=vb_bc, in_=_bcast_rows(b_ext["v"].ap(), 128))
            eps_t = const.tile([GROUPS, 1], F32)
            nc.vector.memset(eps_t, EPS)

            st = {}  # per-batch tile handles

            def emit_A(b):
                """load x, GroupNorm stats + apply -> h"""
                x_sb = xpool.tile([128, CT, N], F32, tag="x", name=f"x{b}")
                h_sb = work.tile([128, CT, N], BF16, tag="h", name=f"h{b}")
                st[b] = {"x": x_sb, "h": h_sb}
                nc.sync.dma_start(out=x_sb, in_=x_ext.ap()[b])
                cstats = small.tile([128, CT, 2, 6], F32, tag="cstats")
                for ct in range(CT):
                    for sg in range(2):
                        nc.vector.bn_stats(
                            out=cstats[:, ct, sg, :],
                            in_=x_sb[:, ct, sg * 512 : (sg + 1) * 512],
                        )
                # bn_stats 6-tuple = (cnt_e, mean_e, cnt*var_e, cnt_o, mean_o,
                # cnt*var_o) over even/odd elements (256 each per 512-chunk).
                # Build per-(channel, chunk) columns a = mean_e + mean_o,
                # b = cnt*var_e + cnt*var_o, c2 = mean_e^2 + mean_o^2, reduce
                # over each group's 32 entries with a 1/64-weighted selector
                # matmul, then mean_g = A, E[x2]_g = B/256 + C2.
                prep = small.tile([128, CT, 2, 3], F32, tag="prep")
                nc.vector.tensor_add(
                    out=prep[:, :, :, 0], in0=cstats[:, :, :, 1], in1=cstats[:, :, :, 4]
                )
                nc.vector.tensor_add(
                    out=prep[:, :, :, 1], in0=cstats[:, :, :, 2], in1=cstats[:, :, :, 5]
                )
                nc.vector.scalar_tensor_tensor(
                    out=cstats[:, :, :, 0],
                    in0=cstats[:, :, :, 1],
                    scalar=0.0,
                    in1=cstats[:, :, :, 1],
                    op0=OP.add,
                    op1=OP.mult,
                )
                nc.vector.scalar_tensor_tensor(
                    out=cstats[:, :, :, 3],
                    in0=cstats[:, :, :, 4],
                    scalar=0.0,
                    in1=cstats[:, :, :, 4],
                    op0=OP.add,
                    op1=OP.mult,
                )
                nc.vector.tensor_add(
                    out=prep[:, :, :, 2], in0=cstats[:, :, :, 0], in1=cstats[:, :, :, 3]
                )
                cb16 = small.tile([128, CT, 2, 3], BF16, tag="cb16")
                nc.vector.tensor_copy(out=cb16, in_=prep)
                # group-reduce matmul (selr carries the 1/64 weight)
                gps = ps2.tile([128, N], F32, tag="ps2", name=f"gps{b}")
                for ct in range(CT):
                    nc.tensor.matmul(
                        out=gps[0:GROUPS, 0:6],
                        lhsT=selr_sb[:, ct, :],
                        rhs=cb16[:, ct, :, :].rearrange("p s f -> p (s f)"),
                        start=(ct == 0),
                        stop=(ct == CT - 1),
                    )
                gsb = small.tile([GROUPS, 6], F32, tag="gsb")
                nc.vector.tensor_copy(out=gsb, in_=gps[0:GROUPS, 0:6])
                gmv = small.tile([GROUPS, 4], F32, tag="gmv")
                nc.vector.tensor_add(out=gmv[:, 0:3], in0=gsb[:, 0:3], in1=gsb[:, 3:6])
                # E[x2] = B/256 + C2 ; var = E[x2] - mean^2
                nc.vector.scalar_tensor_tensor(
                    out=gmv[:, 1:2],
                    in0=gmv[:, 1:2],
                    scalar=1.0 / 256.0,
                    in1=gmv[:, 2:3],
                    op0=OP.mult,
                    op1=OP.add,
                )
                nc.vector.scalar_tensor_tensor(
                    out=gmv[:, 3:4],
                    in0=gmv[:, 0:1],
                    scalar=0.0,
                    in1=gmv[:, 0:1],
                    op0=OP.add,
                    op1=OP.mult,
                )
                nc.vector.tensor_sub(out=gmv[:, 1:2], in0=gmv[:, 1:2], in1=gmv[:, 3:4])
                # rstd = exp(-0.5 * ln(var + eps)) to stay in the exp/ln
                # activation-table set (Sqrt would force a table swap)
                lnv = small.tile([GROUPS, 1], F32, tag="lnv")
                nc.scalar.activation(out=lnv, in_=gmv[:, 1:2], func=AF.Ln, bias=eps_t)
                nc.scalar.activation(out=gmv[:, 1:2], in_=lnv, func=AF.Exp, scale=-0.5)
                gm16 = small.tile([GROUPS, 2], BF16, tag="gm16")
                nc.vector.tensor_copy(out=gm16, in_=gmv[:, 0:2])
                # group-broadcast back to per-channel (mean, rstd)
                cps = ps2.tile([128, N], F32, tag="ps2", name=f"cps{b}")
                for ct in range(CT):
                    nc.tensor.matmul(
                        out=cps[:, ct * 2 : ct * 2 + 2],
                        lhsT=sele_sb[:, ct, :],
                        rhs=gm16,
                        start=True,
                        stop=True,
                    )
                cmv = cps[:, 0:8].rearrange("p (ct s) -> p ct s", s=2)
                csr = small.tile([128, CT], F32, tag="csr")
                nc.vector.tensor_mul(out=csr, in0=cmv[:, :, 1], in1=nsc_sb)
                cb2 = small.tile([128, CT], F32, tag="cb2")
                nc.vector.tensor_mul(out=cb2, in0=cmv[:, :, 0], in1=csr)
                nc.vector.tensor_sub(out=cb2, in0=nbi_sb, in1=cb2)
                for ct in range(CT):
                    nc.vector.tensor_scalar(
                        out=h_sb[:, ct, :],
                        in0=x_sb[:, ct, :],
                        scalar1=csr[:, ct : ct + 1],
                        scalar2=cb2[:, ct : ct + 1],
                        op0=OP.mult,
                        op1=OP.add,
                    )
                if dump and b == 0:
                    nc.gpsimd.dma_start(
                        out=dbg_ext.ap()[0][:, 0:4096],
                        in_=h_sb.rearrange("p a n -> p (a n)"),
                    )

            def conv_units(b):
                """yield per-tile conv work units (q/k ct chains, vT nt chains)"""
                h_sb = st[b]["h"]
                q_sb = work.tile([128, CT, N], BF16, tag="q", name=f"q{b}")
                k_sb = work.tile([128, CT, N], BF16, tag="k", name=f"k{b}")
                vt_sb = work.tile([128, NT, NH, VG], BF16, tag="vt", name=f"vt{b}")
                st[b].update({"q": q_sb, "k": k_sb, "vt": vt_sb})
                nc.vector.memset(vt_sb[:, :, :, D : D + 1], 1.0)

                def qk_unit(nm, dst, ct):
                    def emit():
                        ps = ps2.tile([128, N], F32, tag="ps2", name=f"ps_{nm}{ct}_{b}")
                        for ch in range(NCH):
                            for kt in range(CT):
                                nc.tensor.matmul(
                                    out=ps[:, ch * 512 : (ch + 1) * 512],
                                    lhsT=w_sb[nm][:, kt, ct * 128 : (ct + 1) * 128],
                                    rhs=h_sb[:, kt, ch * 512 : (ch + 1) * 512],
                                    start=(kt == 0),
                                    stop=(kt == CT - 1),
                                )
                        nc.vector.tensor_scalar(
                            out=dst[:, ct, :],
                            in0=ps,
                            scalar1=bias_sb[nm][:, ct : ct + 1],
                            scalar2=None,
                            op0=OP.add,
                        )
                    return emit

                def v_unit(nt):
                    def emit():
                        ps = ps2.tile([128, N], F32, tag="ps2", name=f"ps_v{nt}_{b}")
                        for kt in range(CT):
                            nc.tensor.matmul(
                                out=ps[:, 0:512],
                                lhsT=h_sb[:, kt, nt * 128 : (nt + 1) * 128],
                                rhs=w_sb["v"][:, kt, :],
                                start=(kt == 0),
                                stop=(kt == CT - 1),
                            )
                        psv = ps[:, 0:512].rearrange("p (h d) -> p h d", d=D)
                        if apply_vb:
                            nc.vector.tensor_add(
                                out=vt_sb[:, nt, :, 0:D],
                                in0=psv,
                                in1=vb_bc.rearrange("p (h d) -> p h d", d=D),
                            )
                        else:
                            nc.vector.tensor_copy(out=vt_sb[:, nt, :, 0:D], in_=psv)
                    return emit

                units = []
                for ct in range(CT):
                    units.append(qk_unit("q", q_sb, ct))
                    units.append(qk_unit("k", k_sb, ct))
                for nt in range(0, NT, 2):
                    units.append(v_unit(nt))
                    units.append(v_unit(nt + 1))
                return units

            def proj_units(b):
                """yield per-ct proj+residual+store units"""
                x_sb, att_sb = st[b]["x"], st[b]["att"]
                ov = out_ext.ap()[b]

                def unit(ct):
                    def emit():
                        ps = ps2.tile([128, N], F32, tag="ps2", name=f"ps_p{ct}_{b}")
                        for ch in range(NCH):
                            for kt in range(CT):
                                nc.tensor.matmul(
                                    out=ps[:, ch * 512 : (ch + 1) * 512],
                                    lhsT=w_sb["p"][:, kt, ct * 128 : (ct + 1) * 128],
                                    rhs=att_sb[:, kt, ch * 512 : (ch + 1) * 512],
                                    start=(kt == 0),
                                    stop=(kt == CT - 1),
                                )
                        nc.vector.scalar_tensor_tensor(
                            out=x_sb[:, ct, :],
                            in0=ps,
                            scalar=bias_sb["p"][:, ct : ct + 1],
                            in1=x_sb[:, ct, :],
                            op0=OP.add,
                            op1=OP.add,
                        )
                        nc.sync.dma_start(out=ov[:, ct, :], in_=x_sb[:, ct, :])
                    return emit

                return [unit(ct) for ct in range(CT)]

            def emit_C(b, fillers=()):
                """attention"""
                fillers = list(fillers)

                def fill(k=1):
                    for _ in range(k):
                        if fillers:
                            fillers.pop(0)()

                q_sb, k_sb, vt_sb = st[b]["q"], st[b]["k"], st[b]["vt"]
                att_sb = work.tile(
                    [128, CT, N], BF16, tag="att", bufs=2, name=f"att{b}"
                )
                st[b]["att"] = att_sb
                if dump and b == 0:
                    nc.gpsimd.dma_start(
                        out=dbg_ext.ap()[1][:, 0:4096],
                        in_=q_sb.rearrange("p a n -> p (a n)"),
                    )
                    nc.gpsimd.dma_start(
                        out=dbg_ext.ap()[2][:, 0:4096],
                        in_=k_sb.rearrange("p a n -> p (a n)"),
                    )
                    nc.gpsimd.dma_start(
                        out=dbg_ext.ap()[4][:, 0 : NT * NH * VG],
                        in_=vt_sb.rearrange("p a h g -> p (a h g)"),
                    )
                for hp in range(CT):
                    e_tiles = []
                    for mt in range(NT):
                        psS = ps1.tile([128, 2 * N], F32, tag="ps1", name=f"psS{hp}_{mt}")
                        e_t = epool.tile([128, 2, N], BF16, tag="e", name=f"e{hp}_{mt}")
                        for hi, p0 in ((0, 0), (1, 64)):
                            for ch in range(NCH):
                                nc.tensor.matmul(
                                    out=psS[
                                        :, hi * N + ch * 512 : hi * N + (ch + 1) * 512
                                    ],
                                    lhsT=k_sb[
                                        p0 : p0 + D, hp, mt * 128 : (mt + 1) * 128
                                    ],
                                    rhs=q_sb[p0 : p0 + D, hp, ch * 512 : (ch + 1) * 512],
                                    start=True,
                                    stop=True,
                                    tile_position=(p0, 0),
                                )
                        nc.scalar.activation(
                            out=e_t.rearrange("p a n -> p (a n)"),
                            in_=psS,
                            func=AF.Exp,
                            scale=0.125,
                        )
                        e_tiles.append(e_t)
                        if mt % 2 == 1:
                            fill(1)
                    # compute-engine APs may only start at partition
                    # 0/32/64/96, so the per-head Z rows land as columns of a
                    # single-partition f32 tile, reciprocal'd per pair.
                    zf = small.tile([1, 2 * N], F32, tag="zf", name=f"zf{hp}")
                    for hi, p0 in ((0, 0), (1, 64)):
                        h_ = 2 * hp + hi
                        pso = ps2.tile([128, N], F32, tag="ps2", name=f"psO{hp}_{hi}")
                        for ch in range(NCH):
                            for mt in range(NT):
                                nc.tensor.matmul(
                                    out=pso[0 : D + 1, ch * 512 : (ch + 1) * 512],
                                    lhsT=vt_sb[:, mt, h_, 0 : D + 1],
                                    rhs=e_tiles[mt][:, hi, ch * 512 : (ch + 1) * 512],
                                    start=(mt == 0),
                                    stop=(mt == NT - 1),
                                )
                        nc.vector.tensor_copy(
                            out=att_sb[p0 : p0 + D, hp, :], in_=pso[0:D, :]
                        )
                        nc.vector.tensor_copy(
                            out=zf[0:1, hi * N : (hi + 1) * N],
                            in_=pso[D : D + 1, :],
                        )
                        fill(1)
                    rzf = small.tile([1, 2 * N], F32, tag="rzf", name=f"rzf{hp}")
                    nc.vector.reciprocal_approx_fast(out=rzf, in_=zf)
                    nc.gpsimd.dma_start(
                        out=zdram.ap()[b][2 * hp : 2 * hp + 2], in_=rzf
                    )
                    rzb = small.tile([128, N], BF16, tag="rzb")
                    for hi, p0 in ((0, 0), (1, 64)):
                        nc.sync.dma_start(
                            out=rzb[p0 : p0 + D, :],
                            in_=_bcast_rows(zdram.ap()[b][2 * hp + hi], D),
                        )
                    nc.vector.tensor_mul(
                        out=att_sb[:, hp, :], in0=att_sb[:, hp, :], in1=rzb
                    )
                while fillers:
                    fill(1)
                if dump and b == 0:
                    nc.gpsimd.dma_start(
                        out=dbg_ext.ap()[3][:, 0:4096],
                        in_=att_sb.rearrange("p a n -> p (a n)"),
                    )

            emit_A(0)
            for u in conv_units(0):
                u()
            emit_A(1)
            emit_C(0, fillers=conv_units(1))
            emit_C(1, fillers=proj_units(0))
            for u in proj_units(1):
                u()

    nc.compile()
    return nc


def kernel(x, norm_scale, norm_bias, q_w, q_b, k_w, k_b, v_w, v_b, proj_w, proj_b,
           _dump=False):
    x = np.asarray(x, dtype=np.float32)
    b, c, hh, ww = x.shape
    assert (b, c, hh * ww) == (16, C, N)
    # [b, C, n] -> [b, 128, CT, n] so each SBUF partition loads contiguously
    xr = np.ascontiguousarray(
        x.reshape(b, CT, 128, hh * ww).transpose(0, 2, 1, 3)
    )

    import ml_dtypes

    bf16 = ml_dtypes.bfloat16
    def _wt(w):
        wT = np.asarray(w, np.float32).T.astype(bf16)  # [c' , c]
        return np.ascontiguousarray(
            wT.reshape(CT, 128, C).transpose(1, 0, 2)
        )

    vecs = np.stack(
        [
            np.asarray(v, np.float32).reshape(CT, 128).T
            for v in (norm_scale, norm_bias, q_b, k_b, proj_b)
        ],
        axis=1,
    )  # [128, 5, CT]
    groups_of_p = np.arange(128)[:, None] // GS  # channel-in-tile -> local group
    selr = np.zeros((128, CT, GROUPS), np.float32)
    sele = np.zeros((GROUPS, CT, 128), np.float32)
    for ct in range(CT):
        for p in range(128):
            g = ct * 8 + p // GS
            selr[p, ct, g] = 1.0 / 64.0
            sele[g, ct, p] = 1.0
    import ml_dtypes as _mld

    wts = {
        "qwT": _wt(q_w),
        "kwT": _wt(k_w),
        "vwT": _wt(v_w),
        "pwT": _wt(proj_w),
        "qb": np.ascontiguousarray(np.asarray(q_b, np.float32)),
        "kb": np.ascontiguousarray(np.asarray(k_b, np.float32)),
        "vb": np.ascontiguousarray(np.asarray(v_b, np.float32)),
        "pb": np.ascontiguousarray(np.asarray(proj_b, np.float32)),
        "vecs": np.ascontiguousarray(vecs),
        "selr": np.ascontiguousarray(selr.astype(_mld.bfloat16)),
        "sele": np.ascontiguousarray(sele.astype(_mld.bfloat16)),
    }
    apply_vb = bool(np.any(wts["vb"]))

    nc = build_nc(apply_vb, dump=_dump)
    in_maps = []
    for i in range(N_CORES):
        m = dict(wts)
        m["x"] = np.ascontiguousarray(xr[i * B_PER_CORE : (i + 1) * B_PER_CORE])
        in_maps.append(m)

    res = run_bass_kernel_spmd(nc, in_maps, core_ids=list(range(N_CORES)))
    kernel.last_result = res
    out = np.concatenate([res.results[i]["out"] for i in range(N_CORES)], axis=0)
    # [b, 128, CT, n] -> [b, C, h, w]
    out = out.transpose(0, 2, 1, 3).reshape(b, c, hh, ww)
    return np.ascontiguousarray(out).astype(np.float32)


# revision 25
# speedup vs baseline: 1.0008x; 1.0008x over previous
"""Trainium2 Bass kernel for an AttentionBlock (GroupNorm + single-head-dim
self-attention + proj + residual), data-parallel over batch on 8 NeuronCores.

Reference semantics (per batch element, x: [C=512, H=32, W=32], n = H*W = 1024):
  h   = GroupNorm32(x) * scale + bias
  q   = Wq h + bq ; k = Wk h + bk ; v = Wv h + bv     (1x1 convs, [C, n])
  S_h = q_h^T k_h / sqrt(64)   per head h (8 heads, d=64)
  A_h = softmax(S_h)           (over keys)
  o_h = v_h A_h^T
  y   = x + Wp o + bp

Sharding: batch 16 -> 2 per core, fully independent (no collectives).
"""

import numpy as np

import concourse.bacc as bacc
import concourse.bass as bass
import concourse.tile as tile
from concourse import mybir
from concourse.bass_utils import run_bass_kernel_spmd

F32 = mybir.dt.float32
F32R = mybir.dt.float32r
BF16 = mybir.dt.bfloat16
AF = mybir.ActivationFunctionType
OP = mybir.AluOpType

C = 512
NH = 8
D = 64
N = 1024
GROUPS = 32
GS = C // GROUPS  # 16 channels per group
EPS = 1e-5
B_PER_CORE = 2
N_CORES = 8

CT = 4   # c tiles of 128
NT = 8   # n tiles of 128
NCH = 2  # n chunks of 512
VG = 66  # vT per-head group stride (64 data + 1 ones + 1 pad)

E_BUFS = 12


def _bcast_rows(row_ap, parts):
    """Broadcast a single-row (DRAM) AP across `parts` partitions."""
    ap = [[0, parts]] + [list(d) for d in row_ap.ap]
    return bass.AP(tensor=row_ap.tensor, offset=row_ap.offset, ap=ap)


def build_nc(apply_vb, dump=False):
    nc = bacc.Bacc()

    x_ext = nc.declare_dram_parameter("x", [B_PER_CORE, 128, CT, N], F32, isOutput=False)
    w_ext = {}
    b_ext = {}
    for nm in ("q", "k", "v", "p"):
        w_ext[nm] = nc.declare_dram_parameter(f"{nm}wT", [128, CT, C], BF16, isOutput=False)
        b_ext[nm] = nc.declare_dram_parameter(f"{nm}b", [C], F32, isOutput=False)
    # packed per-channel vectors, pre-transposed host-side:
    # [128, 5, CT] = (nsc, nbi, qb, kb, pb) x c-tile
    vecs_ext = nc.declare_dram_parameter("vecs", [128, 5, CT], F32, isOutput=False)
    selr_ext = nc.declare_dram_parameter("selr", [128, CT, GROUPS], BF16, isOutput=False)
    sele_ext = nc.declare_dram_parameter("sele", [GROUPS, CT, 128], BF16, isOutput=False)
    out_ext = nc.declare_dram_parameter("out", [B_PER_CORE, 128, CT, N], F32, isOutput=True)

    zdram = nc.dram_tensor("zscratch", [B_PER_CORE, NH, N], BF16)
    dbg_ext = None
    if dump:
        dbg_ext = nc.declare_dram_parameter("dbg", [10, 128, 4352], F32, isOutput=True)

    with tile.TileContext(nc) as tc:
        with (
            tc.tile_pool(name="const", bufs=1) as const,
            tc.tile_pool(name="work", bufs=2) as work,
            tc.tile_pool(name="xpool", bufs=2) as xpool,
            tc.tile_pool(name="epool", bufs=E_BUFS) as epool,
            tc.tile_pool(name="small", bufs=2) as small,
            tc.tile_pool(name="ps1", bufs=1, space="PSUM") as ps1,
            tc.tile_pool(name="ps2", bufs=2, space="PSUM") as ps2,
        ):
            # ---- persistent weight / bias tiles -------------------------
            w_sb = {}
            for nm, eng in (("q", nc.sync), ("k", nc.sync), ("v", nc.sync), ("p", nc.sync)):
                w_sb[nm] = const.tile([128, CT, C], BF16, name=f"w_{nm}")
                eng.dma_start(out=w_sb[nm], in_=w_ext[nm].ap())
            vecs_sb = const.tile([128, 5, CT], F32)
            nc.sync.dma_start(out=vecs_sb, in_=vecs_ext.ap())
            nsc_sb = vecs_sb[:, 0, :]
            nbi_sb = vecs_sb[:, 1, :]
            bias_sb = {"q": vecs_sb[:, 2, :], "k": vecs_sb[:, 3, :], "p": vecs_sb[:, 4, :]}
            selr_sb = const.tile([128, CT, GROUPS], BF16)
            nc.sync.dma_start(out=selr_sb, in_=selr_ext.ap())
            sele_sb = const.tile([GROUPS, CT, 128], BF16)
            nc.sync.dma_start(out=sele_sb, in_=sele_ext.ap())
            vb_bc = None
            if apply_vb:
                vb_bc = const.tile([128, C], F32)
                nc.sync.dma_start(out=vb_bc, in_=_bcast_rows(b_ext["v"].ap(), 128))
            eps_t = const.tile([GROUPS, 1], F32)
            nc.vector.memset(eps_t, EPS)

            st = {}  # per-batch tile handles

            def emit_A(b):
                """load x, GroupNorm stats + apply -> h"""
                x_sb = xpool.tile([128, CT, N], F32, tag="x", name=f"x{b}")
                h_sb = work.tile([128, CT, N], BF16, tag="h", bufs=1, name=f"h{b}")
                st[b] = {"x": x_sb, "h": h_sb}
                nc.sync.dma_start(out=x_sb, in_=x_ext.ap()[b])
                cstats = small.tile([128, CT, 2, 6], F32, tag="cstats", bufs=1)
                for ct in range(CT):
                    for sg in range(2):
                        nc.vector.bn_stats(
                            out=cstats[:, ct, sg, :],
                            in_=x_sb[:, ct, sg * 512 : (sg + 1) * 512],
                        )
                # bn_stats 6-tuple = (cnt_e, mean_e, cnt*var_e, cnt_o, mean_o,
                # cnt*var_o) over even/odd elements (256 each per 512-chunk).
                # Build per-(channel, chunk) columns a = mean_e + mean_o,
                # b = cnt*var_e + cnt*var_o, c2 = mean_e^2 + mean_o^2, reduce
                # over each group's 32 entries with a 1/64-weighted selector
                # matmul, then mean_g = A, E[x2]_g = B/256 + C2.
                prep = small.tile([128, CT, 2, 3], F32, tag="prep", bufs=1)
                nc.vector.tensor_add(
                    out=prep[:, :, :, 0], in0=cstats[:, :, :, 1], in1=cstats[:, :, :, 4]
                )
                nc.vector.tensor_add(
                    out=prep[:, :, :, 1], in0=cstats[:, :, :, 2], in1=cstats[:, :, :, 5]
                )
                nc.vector.scalar_tensor_tensor(
                    out=cstats[:, :, :, 0],
                    in0=cstats[:, :, :, 1],
                    scalar=0.0,
                    in1=cstats[:, :, :, 1],
                    op0=OP.add,
                    op1=OP.mult,
                )
                nc.vector.scalar_tensor_tensor(
                    out=cstats[:, :, :, 3],
                    in0=cstats[:, :, :, 4],
                    scalar=0.0,
                    in1=cstats[:, :, :, 4],
                    op0=OP.add,
                    op1=OP.mult,
                )
                nc.vector.tensor_add(
                    out=prep[:, :, :, 2], in0=cstats[:, :, :, 0], in1=cstats[:, :, :, 3]
                )
                cb16 = small.tile([128, CT, 2, 3], BF16, tag="cb16")
                nc.vector.tensor_copy(out=cb16, in_=prep)
                # group-reduce matmul (selr carries the 1/64 weight)
                gps = ps2.tile([128, N], F32, tag="ps2", name=f"gps{b}")
                for ct in range(CT):
                    nc.tensor.matmul(
                        out=gps[0:GROUPS, 0:6],
                        lhsT=selr_sb[:, ct, :],
                        rhs=cb16[:, ct, :, :].rearrange("p s f -> p (s f)"),
                        start=(ct == 0),
                        stop=(ct == CT - 1),
                    )
                gsb = small.tile([GROUPS, 6], F32, tag="gsb")
                nc.vector.tensor_copy(out=gsb, in_=gps[0:GROUPS, 0:6])
                gmv = small.tile([GROUPS, 4], F32, tag="gmv")
                nc.vector.tensor_add(out=gmv[:, 0:3], in0=gsb[:, 0:3], in1=gsb[:, 3:6])
                # E[x2] = B/256 + C2 ; var = E[x2] - mean^2
                nc.vector.scalar_tensor_tensor(
                    out=gmv[:, 1:2],
                    in0=gmv[:, 1:2],
                    scalar=1.0 / 256.0,
                    in1=gmv[:, 2:3],
                    op0=OP.mult,
                    op1=OP.add,
                )
                nc.vector.scalar_tensor_tensor(
                    out=gmv[:, 3:4],
                    in0=gmv[:, 0:1],
                    scalar=0.0,
                    in1=gmv[:, 0:1],
                    op0=OP.add,
                    op1=OP.mult,
                )
                nc.vector.tensor_sub(out=gmv[:, 1:2], in0=gmv[:, 1:2], in1=gmv[:, 3:4])
                # rstd = exp(-0.5 * ln(var + eps)) to stay in the exp/ln
                # activation-table set (Sqrt would force a table swap)
                lnv = small.tile([GROUPS, 1], F32, tag="lnv")
                nc.scalar.activation(out=lnv, in_=gmv[:, 1:2], func=AF.Ln, bias=eps_t)
                nc.scalar.activation(out=gmv[:, 1:2], in_=lnv, func=AF.Exp, scale=-0.5)
                gm16 = small.tile([GROUPS, 2], BF16, tag="gm16")
                nc.vector.tensor_copy(out=gm16, in_=gmv[:, 0:2])
                # group-broadcast back to per-channel (mean, rstd)
                cps = ps2.tile([128, N], F32, tag="ps2", name=f"cps{b}")
                for ct in range(CT):
                    nc.tensor.matmul(
                        out=cps[:, ct * 2 : ct * 2 + 2],
                        lhsT=sele_sb[:, ct, :],
                        rhs=gm16,
                        start=True,
                        stop=True,
                    )
                cmv = cps[:, 0:8].rearrange("p (ct s) -> p ct s", s=2)
                csr = small.tile([128, CT], F32, tag="csr")
                nc.vector.tensor_mul(out=csr, in0=cmv[:, :, 1], in1=nsc_sb)
                cb2 = small.tile([128, CT], F32, tag="cb2")
                nc.vector.tensor_mul(out=cb2, in0=cmv[:, :, 0], in1=csr)
                nc.vector.tensor_sub(out=cb2, in0=nbi_sb, in1=cb2)
                for ct in range(CT):
                    nc.vector.tensor_scalar(
                        out=h_sb[:, ct, :],
                        in0=x_sb[:, ct, :],
                        scalar1=csr[:, ct : ct + 1],
                        scalar2=cb2[:, ct : ct + 1],
                        op0=OP.mult,
                        op1=OP.add,
                    )
                if dump and b == 0:
                    nc.gpsimd.dma_start(
                        out=dbg_ext.ap()[0][:, 0:4096],
                        in_=h_sb.rearrange("p a n -> p (a n)"),
                    )

            def conv_units(b):
                """yield per-tile conv work units (q/k ct chains, vT nt chains)"""
                h_sb = st[b]["h"]
                q_sb = work.tile([128, CT, N], BF16, tag="q", name=f"q{b}")
                k_sb = work.tile([128, CT, N], BF16, tag="k", name=f"k{b}")
                vt_sb = work.tile([128, NT, NH, VG], BF16, tag="vt", name=f"vt{b}")
                st[b].update({"q": q_sb, "k": k_sb, "vt": vt_sb})
                nc.vector.memset(vt_sb[:, :, :, D : D + 1], 1.0)

                def qk_unit(nm, dst, ct):
                    def emit():
                        ps = ps2.tile([128, N], F32, tag="ps2", name=f"ps_{nm}{ct}_{b}")
                        for ch in range(NCH):
                            for kt in range(CT):
                                nc.tensor.matmul(
                                    out=ps[:, ch * 512 : (ch + 1) * 512],
                                    lhsT=w_sb[nm][:, kt, ct * 128 : (ct + 1) * 128],
                                    rhs=h_sb[:, kt, ch * 512 : (ch + 1) * 512],
                                    start=(kt == 0),
                                    stop=(kt == CT - 1),
                                )
                        nc.vector.tensor_scalar(
                            out=dst[:, ct, :],
                            in0=ps,
                            scalar1=bias_sb[nm][:, ct : ct + 1],
                            scalar2=None,
                            op0=OP.add,
                        )
                    return emit

                def v_unit(nt):
                    def emit():
                        ps = ps2.tile([128, N], F32, tag="ps2", name=f"ps_v{nt}_{b}")
                        for kt in range(CT):
                            nc.tensor.matmul(
                                out=ps[:, 0:512],
                                lhsT=h_sb[:, kt, nt * 128 : (nt + 1) * 128],
                                rhs=w_sb["v"][:, kt, :],
                                start=(kt == 0),
                                stop=(kt == CT - 1),
                            )
                        psv = ps[:, 0:512].rearrange("p (h d) -> p h d", d=D)
                        if apply_vb:
                            nc.vector.tensor_add(
                                out=vt_sb[:, nt, :, 0:D],
                                in0=psv,
                                in1=vb_bc.rearrange("p (h d) -> p h d", d=D),
                            )
                        else:
                            nc.vector.tensor_copy(out=vt_sb[:, nt, :, 0:D], in_=psv)
                    return emit

                units = []
                for ct in range(CT):
                    units.append(qk_unit("q", q_sb, ct))
                    units.append(qk_unit("k", k_sb, ct))
                for nt in range(0, NT, 2):
                    units.append(v_unit(nt))
                    units.append(v_unit(nt + 1))
                return units

            def proj_units(b):
                """yield per-ct proj+residual+store units"""
                x_sb, att_sb = st[b]["x"], st[b]["att"]
                ov = out_ext.ap()[b]

                def unit(ct):
                    def emit():
                        ps = ps2.tile([128, N], F32, tag="ps2", name=f"ps_p{ct}_{b}")
                        for ch in range(NCH):
                            for kt in range(CT):
                                nc.tensor.matmul(
                                    out=ps[:, ch * 512 : (ch + 1) * 512],
                                    lhsT=w_sb["p"][:, kt, ct * 128 : (ct + 1) * 128],
                                    rhs=att_sb[:, kt, ch * 512 : (ch + 1) * 512],
                                    start=(kt == 0),
                                    stop=(kt == CT - 1),
                                )
                        nc.vector.scalar_tensor_tensor(
                            out=x_sb[:, ct, :],
                            in0=ps,
                            scalar=bias_sb["p"][:, ct : ct + 1],
                            in1=x_sb[:, ct, :],
                            op0=OP.add,
                            op1=OP.add,
                        )
                        nc.sync.dma_start(out=ov[:, ct, :], in_=x_sb[:, ct, :])
                    return emit

                return [unit(ct) for ct in range(CT)]

            fillq = []

            def fill(k=1):
                for _ in range(k):
                    if fillq:
                        fillq.pop(0)()

            def prep_att(b):
                att_sb = work.tile(
                    [128, CT, N], BF16, tag="att", bufs=2, name=f"att{b}"
                )
                st[b]["att"] = att_sb

            def emit_pair(b, hp):
                """one head-pair of attention: S^T matmuls + exp + out/Z
                matmuls + softmax normalization"""
                q_sb, k_sb, vt_sb = st[b]["q"], st[b]["k"], st[b]["vt"]
                att_sb = st[b]["att"]
                e_tiles = []
                for mt in range(NT):
                    psS = ps1.tile(
                        [128, 2 * N], F32, tag="ps1", name=f"psS{b}_{hp}_{mt}"
                    )
                    e_t = epool.tile(
                        [128, 2, N], BF16, tag="e", name=f"e{b}_{hp}_{mt}"
                    )
                    for hi, p0 in ((0, 0), (1, 64)):
                        for ch in range(NCH):
                            nc.tensor.matmul(
                                out=psS[
                                    :, hi * N + ch * 512 : hi * N + (ch + 1) * 512
                                ],
                                lhsT=k_sb[p0 : p0 + D, hp, mt * 128 : (mt + 1) * 128],
                                rhs=q_sb[p0 : p0 + D, hp, ch * 512 : (ch + 1) * 512],
                                start=True,
                                stop=True,
                                tile_position=(p0, 0),
                            )
                    nc.scalar.activation(
                        out=e_t.rearrange("p a n -> p (a n)"),
                        in_=psS,
                        func=AF.Exp,
                        scale=0.125,
                    )
                    e_tiles.append(e_t)
                    fill(1)
                # compute-engine APs may only start at partition 0/32/64/96,
                # so the per-head Z rows land as columns of a single-partition
                # f32 tile, reciprocal'd (in place) per pair.
                zf = small.tile([1, 2 * N], F32, tag="zf", name=f"zf{b}_{hp}")
                for hi, p0 in ((0, 0), (1, 64)):
                    h_ = 2 * hp + hi
                    pso = ps2.tile([128, N], F32, tag="ps2", name=f"psO{b}_{hp}_{hi}")
                    for ch in range(NCH):
                        for mt in range(NT):
                            nc.tensor.matmul(
                                out=pso[0 : D + 1, ch * 512 : (ch + 1) * 512],
                                lhsT=vt_sb[:, mt, h_, 0 : D + 1],
                                rhs=e_tiles[mt][:, hi, ch * 512 : (ch + 1) * 512],
                                start=(mt == 0),
                                stop=(mt == NT - 1),
                            )
                    nc.vector.tensor_copy(
                        out=att_sb[p0 : p0 + D, hp, :], in_=pso[0:D, :]
                    )
                    nc.vector.tensor_copy(
                        out=zf[0:1, hi * N : (hi + 1) * N], in_=pso[D : D + 1, :]
                    )
                    fill(1)
                nc.vector.reciprocal_approx_fast(out=zf, in_=zf)
                nc.gpsimd.dma_start(out=zdram.ap()[b][2 * hp : 2 * hp + 2], in_=zf)
                rzb = small.tile([128, N], BF16, tag="rzb", name=f"rzb{b}_{hp}")
                for hi, p0 in ((0, 0), (1, 64)):
                    nc.sync.dma_start(
                        out=rzb[p0 : p0 + D, :],
                        in_=_bcast_rows(zdram.ap()[b][2 * hp + hi], D),
                    )
                nc.vector.tensor_mul(
                    out=att_sb[:, hp, :], in0=att_sb[:, hp, :], in1=rzb
                )

            emit_A(0)
            for u in conv_units(0):
                u()
            emit_A(1)
            prep_att(0)
            prep_att(1)
            emit_pair(0, 0)
            fillq.extend(conv_units(1))
            emit_pair(0, 1)
            emit_pair(1, 0)
            emit_pair(0, 2)
            emit_pair(1, 1)
            emit_pair(0, 3)
            fillq.extend(proj_units(0))
            emit_pair(1, 2)
            emit_pair(1, 3)
            while fillq:
                fill(1)
            for u in proj_units(1):
                u()

    nc.compile()
    return nc


def kernel(x, norm_scale, norm_bias, q_w, q_b, k_w, k_b, v_w, v_b, proj_w, proj_b,
           _dump=False):
    x = np.asarray(x, dtype=np.float32)
    b, c, hh, ww = x.shape
    assert (b, c, hh * ww) == (16, C, N)
    # [b, C, n] -> [b, 128, CT, n] so each SBUF partition loads contiguously
    xr = np.ascontiguousarray(
        x.reshape(b, CT, 128, hh * ww).transpose(0, 2, 1, 3)
    )

    import ml_dtypes

    bf16 = ml_dtypes.bfloat16
    def _wt(w):
        wT = np.asarray(w, np.float32).T.astype(bf16)  # [c' , c]
        return np.ascontiguousarray(
            wT.reshape(CT, 128, C).transpose(1, 0, 2)
        )

    vecs = np.stack(
        [
            np.asarray(v, np.float32).reshape(CT, 128).T
            for v in (norm_scale, norm_bias, q_b, k_b, proj_b)
        ],
        axis=1,
    )  # [128, 5, CT]
    groups_of_p = np.arange(128)[:, None] // GS  # channel-in-tile -> local group
    selr = np.zeros((128, CT, GROUPS), np.float32)
    sele = np.zeros((GROUPS, CT, 128), np.float32)
    for ct in range(CT):
        for p in range(128):
            g = ct * 8 + p // GS
            selr[p, ct, g] = 1.0 / 64.0
            sele[g, ct, p] = 1.0
    import ml_dtypes as _mld

    wts = {
        "qwT": _wt(q_w),
        "kwT": _wt(k_w),
        "vwT": _wt(v_w),
        "pwT": _wt(proj_w),
        "qb": np.ascontiguousarray(np.asarray(q_b, np.float32)),
        "kb": np.ascontiguousarray(np.asarray(k_b, np.float32)),
        "vb": np.ascontiguousarray(np.asarray(v_b, np.float32)),
        "pb": np.ascontiguousarray(np.asarray(proj_b, np.float32)),
        "vecs": np.ascontiguousarray(vecs),
        "selr": np.ascontiguousarray(selr.astype(_mld.bfloat16)),
        "sele": np.ascontiguousarray(sele.astype(_mld.bfloat16)),
    }
    apply_vb = bool(np.any(wts["vb"]))

    nc = build_nc(apply_vb, dump=_dump)
    in_maps = []
    for i in range(N_CORES):
        m = dict(wts)
        m["x"] = np.ascontiguousarray(xr[i * B_PER_CORE : (i + 1) * B_PER_CORE])
        in_maps.append(m)

    res = run_bass_kernel_spmd(nc, in_maps, core_ids=list(range(N_CORES)))
    kernel.last_result = res
    out = np.concatenate([res.results[i]["out"] for i in range(N_CORES)], axis=0)
    # [b, 128, CT, n] -> [b, C, h, w]
    out = out.transpose(0, 2, 1, 3).reshape(b, c, hh, ww)
    return np.ascontiguousarray(out).astype(np.float32)
